# revision 28
# baseline (speedup 1.0000x reference)
"""Trainium2 Bass kernel for nn_MultiHeadAttention (GQA + RoPE + causal softmax).

Problem (hardcoded): B=4, T=2048, C=2048, n_head=16, n_kv_head=4, head_dim=128,
fp32 in/out, rope base 10000, torch-Linear style projections (x @ W.T).

Sharding: 8 cores = (4 batches) x (2 query shards). Each core handles one batch
and 1024 query rows picked as interleaved 128-row blocks (core parity 0 takes
even blocks, parity 1 odd blocks) so both cores of a batch run an identical
instruction stream (SPMD) with identical causal work. K/V are computed for the
full sequence on both cores of a pair. No collectives; host gathers outputs.

All matmuls run in float32r (TF32-like, ~1.5e-4 rel err). Device layout is
transposed ([feature, token]) so every matmul contraction is on partitions.

RoPE trick: the head_dim rows of Wq/Wk (and the trig tables) are permuted on
the host so each rotate-half pair (i, i+64) lands 16 partitions apart inside
one 32-partition quadrant. rotate_half then is a single DVE stream_shuffle
(quadrant-local 16<->16 swap) instead of cross-partition DMA copies. Dot
products q.k are invariant to the shared permutation.

Attention is flash-style per (kv-group, head-pair, 256-query block) with the
softmax-denominator and P.V matmuls deferred two chunks behind the score
matmuls so the PE never waits on the mask(DVE)+exp(ACT) latency. The
normalized output is written straight into an SBUF-resident y tile that the
output projection consumes directly (no DRAM roundtrip for y).
"""

import sys
import math
from collections import deque

sys.path.insert(0, "/opt/trn_rl_repo")

import numpy as np

import concourse.bacc as bacc
import concourse.mybir as mybir
import concourse.tile as tile
from concourse.bass_utils import run_bass_kernel_spmd

F32 = mybir.dt.float32
F32R = mybir.dt.float32r
AF = mybir.ActivationFunctionType

B, T, C = 4, 2048, 2048
NH, NKV, HD = 16, 4, 128
NREP = NH // NKV              # 4 q-heads per kv head
ROPE_BASE = 10000.0
R = T // 2                    # 1024 query rows per core
NCC = C // 128                # 16 contraction chunks
NQB = R // 128                # 8 local query blocks per core
NPAIR = NQB // 2              # 4 pair-blocks of 256 queries
MASK_NEG = -30000.0
SWAP_MASK = list(range(16, 32)) + list(range(16))
LOOKAHEAD = 2                 # chunks of den/PV deferral behind S matmuls


def _build_nc(nrep=1):
    nc = bacc.Bacc(trn_type="TRN2", name="mha_gqa_rope")

    xT = nc.dram_tensor("xT", [C, T], F32R, kind="ExternalInput")
    xqT = nc.dram_tensor("xqT", [C, R], F32R, kind="ExternalInput")
    wqT = nc.dram_tensor("wqT", [C, C], F32R, kind="ExternalInput")
    wkT = nc.dram_tensor("wkT", [C, NKV * HD], F32R, kind="ExternalInput")
    wvT = nc.dram_tensor("wvT", [C, NKV * HD], F32R, kind="ExternalInput")
    woT = nc.dram_tensor("woT", [C, C], F32R, kind="ExternalInput")
    cosq = nc.dram_tensor("cosq", [HD, R], F32, kind="ExternalInput")
    sinq = nc.dram_tensor("sinq", [HD, R], F32, kind="ExternalInput")
    cosk = nc.dram_tensor("cosk", [HD, T], F32, kind="ExternalInput")
    sink = nc.dram_tensor("sink", [HD, T], F32, kind="ExternalInput")
    maskadd = nc.dram_tensor("maskadd", [128, 4 * 512], F32, kind="ExternalInput")
    ones_d = nc.dram_tensor("ones_d", [128, 128], F32R, kind="ExternalInput")
    outT = nc.dram_tensor("outT", [C, R], F32, kind="ExternalOutput")

    with tile.TileContext(nc) as tc:
        with tc.tile_pool(name="dscr", bufs=1, space="DRAM") as dscr, \
             tc.tile_pool(name="const", bufs=1) as constp:
            qscr = dscr.tile([C, R], F32R)

            ones_s = constp.tile([128, 128], F32R)
            nc.sync.dma_start(out=ones_s[:], in_=ones_d.ap())

            for _rep in range(nrep):
                # K/V weights + key trig prefetched on the ACT (scalar) DMA
                # queue; transfers complete during stage Q. Explicit alloc /
                # release: these pools close after stage KV while kv_res
                # (opened later) persists into the attention stage.
                xt0p = tc.alloc_tile_pool(name="xt0", bufs=1, side="right")
                kvwp = tc.alloc_tile_pool(name="kvw", bufs=1, side="right")
                ktrigp = tc.alloc_tile_pool(name="ktrig", bufs=1, side="right")
                if True:
                    wk_s = kvwp.tile([128, NCC, NKV * HD], F32R, tag="wk")
                    wv_s = kvwp.tile([128, NCC, NKV * HD], F32R, tag="wv")
                    cosk_s = ktrigp.tile([HD, T], F32)
                    sink_s = ktrigp.tile([HD, T], F32)

                    # ------------- Stage Q: Q'^T = rope(WqT.T @ xqT) -> qscr --
                    with tc.tile_pool(name="xq", bufs=1) as xqp, \
                         tc.tile_pool(name="wq", bufs=2) as wqp, \
                         tc.tile_pool(name="qtrig", bufs=1) as qtrigp, \
                         tc.tile_pool(name="qrope", bufs=2) as qrp, \
                         tc.tile_pool(name="qpsum", bufs=3, space="PSUM") as qps:
        # Loads split across both HWDGE queues: sync carries wq
                        # strips + even xq chunks; ACT carries trig + odd xq
                        # chunks, then the stage-KV prefetches (wk/wv/cosk/sink).
                        xq_s = xqp.tile([128, NCC, R], F32R)
                        wq_strips = []

                        def load_wq(qc):
                            wq_strip = wqp.tile([128, NCC, 128], F32R, tag="wq",
                                                name=f"wq_strip{qc}")
                            nc.sync.dma_start(
                                out=wq_strip[:],
                                in_=wqT.ap()[:, qc * 128:(qc + 1) * 128].rearrange(
                                    "(c p) m -> p c m", p=128
                                ),
                            )
                            wq_strips.append(wq_strip)

                        def load_xq(c, eng):
                            eng.dma_start(
                                out=xq_s[:, c, :],
                                in_=xqT.ap()[c * 128:(c + 1) * 128, :],
                            )

                        cosq_s = qtrigp.tile([HD, R], F32)
                        nc.scalar.dma_start(out=cosq_s[:], in_=cosq.ap())
                        sinq_s = qtrigp.tile([HD, R], F32)
                        nc.scalar.dma_start(out=sinq_s[:], in_=sinq.ap())
                        load_wq(0)
                        for c in range(NCC):
                            load_xq(c, nc.sync if c % 2 == 0 else nc.scalar)
                            if c == 3:
                                load_wq(1)
                        # stage-KV prefetches ride the ACT queue from here
                        nc.scalar.dma_start(
                            out=wk_s[:], in_=wkT.ap().rearrange("(c p) k -> p c k", p=128)
                        )
                        nc.scalar.dma_start(
                            out=wv_s[:], in_=wvT.ap().rearrange("(c p) k -> p c k", p=128)
                        )
                        nc.scalar.dma_start(out=cosk_s[:], in_=cosk.ap())
                        nc.scalar.dma_start(out=sink_s[:], in_=sink.ap())
                        for qc in range(NH):  # 16 head-chunks of Q output dims
                            if qc + 2 < NH:
                                load_wq(qc + 2)
                            wq_strip = wq_strips[qc]
                            psqs = []
                            for rb in range(R // 512):
                                psq = qps.tile([128, 512], F32, tag="psq",
                                               name=f"psq{qc}_{rb}")
                                psqs.append(psq)
                            for c in range(NCC):
                                for rb in range(R // 512):
                                    nc.tensor.matmul(
                                        psqs[rb][:],
                                        wq_strip[:, c, :],
                                        xq_s[:, c, rb * 512:(rb + 1) * 512],
                                        start=(c == 0),
                                        stop=(c == NCC - 1),
                                    )
                            # rope via quadrant-local stream_shuffle (DVE only)
                            for rb in range(R // 512):
                                sl = slice(rb * 512, (rb + 1) * 512)
                                psq = psqs[rb]
                                rot = qrp.tile([128, 512], F32, tag="rot")
                                nc.vector.stream_shuffle(rot[:], psq[:], SWAP_MASK)
                                t1 = qrp.tile([128, 512], F32, tag="t1")
                                nc.vector.tensor_mul(t1[:], psq[:], cosq_s[:, sl])
                                nc.vector.tensor_mul(rot[:], rot[:], sinq_s[:, sl])
                                qf = qrp.tile([128, 512], F32R, tag="qf")
                                nc.vector.tensor_add(qf[:], t1[:], rot[:])
                                nc.gpsimd.dma_start(
                                    out=qscr[qc * 128:(qc + 1) * 128, sl], in_=qf[:]
                                )

                    # Causal-mask table: load early on the sync queue so it's
                    # resident well before the first attention mask-add.
                    cmaskp = tc.alloc_tile_pool(name="cmask", bufs=1)
                    mask_s = cmaskp.tile([128, 4 * 512], F32)
                    nc.sync.dma_start(out=mask_s[:], in_=maskadd.ap())

                    # First x block in a dedicated region (disjoint from the
                    # stage-Q pools) so its load isn't WAR-gated on stage Q.
                    xt0_tile = xt0p.tile([128, NCC, 256], F32R, tag="xt0", bufs=1)
                    nc.sync.dma_start(
                        out=xt0_tile[:],
                        in_=xT.ap()[:, 0:256].rearrange("(c p) t -> p c t", p=128),
                    )

                    # ------------- Stage KV ----------------------------------
                    with tc.tile_pool(name="kv_res", bufs=1) as kvres:
                        kT_s = kvres.tile([128, NKV, T], F32R)   # [d, g, t]
                        v_s = kvres.tile([128, T // 128, NKV * HD], F32R)

                        with tc.tile_pool(name="xt", bufs=2) as xtp, \
                             tc.tile_pool(name="krope", bufs=3) as krp, \
                             tc.tile_pool(name="kpsum", bufs=3, space="PSUM") as kps, \
                             tc.tile_pool(name="vpsum", bufs=2, space="PSUM") as vps:
                            for tb in range(T // 256):
                                if tb == 0:
                                    xt = xt0_tile
                                else:
                                    xt = xtp.tile([128, NCC, 256], F32R, tag="xt")
                                    nc.sync.dma_start(
                                        out=xt[:],
                                        in_=xT.ap()[:, tb * 256:(tb + 1) * 256].rearrange(
                                            "(c p) t -> p c t", p=128
                                        ),
                                    )
                                for g in range(NKV):
                                    psk = kps.tile([128, 256], F32, tag="psk")
                                    for c in range(NCC):
                                        nc.tensor.matmul(
                                            psk[:],
                                            wk_s[:, c, g * 128:(g + 1) * 128],
                                            xt[:, c, :],
                                            start=(c == 0),
                                            stop=(c == NCC - 1),
                                        )
                                    sl = slice(tb * 256, (tb + 1) * 256)
                                    rot = krp.tile([128, 256], F32, tag="krot")
                                    nc.vector.stream_shuffle(rot[:], psk[:], SWAP_MASK)
                                    t1 = krp.tile([128, 256], F32, tag="kt1")
                                    nc.vector.tensor_mul(t1[:], psk[:], cosk_s[:, sl])
                                    nc.vector.tensor_mul(rot[:], rot[:], sink_s[:, sl])
                                    nc.vector.tensor_add(kT_s[:, g, sl], t1[:], rot[:])
                                for ti in range(2):
                                    tchunk = tb * 2 + ti
                                    psv = vps.tile([128, NKV * HD], F32, tag="psv")
                                    for c in range(NCC):
                                        nc.tensor.matmul(
                                            psv[:],
                                            xt[:, c, ti * 128:(ti + 1) * 128],
                                            wv_s[:, c, :],
                                            start=(c == 0),
                                            stop=(c == NCC - 1),
                                        )
                                    nc.scalar.copy(v_s[:, tchunk, :], psv[:])

                        # qp for pair-block 0 reuses the xt0 region (free after
                        # t-block 0): its load runs during stage KV instead of
                        # being WAR-gated on the whole KV x ring.
                        qp0 = xt0p.tile([128, NH, 256], F32R, tag="xt0", bufs=1,
                                        name="qp0")
                        nc.sync.dma_start(
                            out=qp0[:],
                            in_=qscr[:, 0:256].rearrange("(h p) q -> p h q", p=128),
                        )

                        # K/V weights + key trig no longer needed
                        ktrigp.release()
                        kvwp.release()

                        # ---------- Stage C+D: attention + out proj ----------
                        with tc.tile_pool(name="ybuf", bufs=1) as ybufp:
                            y_s = ybufp.tile([128, NH, R], F32R)  # resident y^T

                            with tc.tile_pool(name="qp", bufs=2) as qpp, \
                                 tc.tile_pool(name="ptile", bufs=4) as ppp, \
                                 tc.tile_pool(name="small", bufs=2) as smallp, \
                                 tc.tile_pool(name="spsum", bufs=3, space="PSUM") as sps, \
                                 tc.tile_pool(name="opsum", bufs=2, space="PSUM") as ops, \
                                 tc.tile_pool(name="dpsum", bufs=2, space="PSUM") as dps:
                                deferred = deque()

                                def emit(fn):
                                    deferred.append(fn)
                                    while len(deferred) > LOOKAHEAD:
                                        deferred.popleft()()

                                for jj in range(NPAIR):
                                    if jj == 0:
                                        qp = qp0
                                    else:
                                        qp = qpp.tile([128, NH, 256], F32R, tag="qp")
                                        nc.sync.dma_start(
                                            out=qp[:],
                                            in_=qscr[:, jj * 256:(jj + 1) * 256].rearrange(
                                                "(h p) q -> p h q", p=128
                                            ),
                                        )
                                    qp_flat = qp[:].rearrange("p h q -> p (h q)")
                                    nchunks = 4 * jj + 4
                                    for g in range(NKV):
                                        for hp in range(NREP // 2):
                                            hh = g * NREP + hp * 2
                                            den = dps.tile([1, 512], F32, tag="den")
                                            po = ops.tile([128, 512], F32, tag="po")
                                            for cc in range(nchunks):
                                                pss = sps.tile([128, 512], F32, tag="pss")
                                                nc.tensor.matmul(
                                                    pss[:],
                                                    kT_s[:, g, cc * 128:(cc + 1) * 128],
                                                    qp_flat[:, hh * 256:(hh + 2) * 256],
                                                    start=True,
                                                    stop=True,
                                                )
                                                if cc >= 4 * jj:
                                                    moff = (cc - 4 * jj) * 512
                                                    nc.vector.tensor_add(
                                                        pss[:], pss[:],
                                                        mask_s[:, moff:moff + 512],
                                                    )
                                                pt = ppp.tile([128, 512], F32R, tag="pt")
                                                nc.scalar.activation(pt[:], pss[:], AF.Exp)

                                                def denpv(den=den, po=po, pt=pt,
                                                          cc=cc, nchunks=nchunks, g=g):
                                                    nc.tensor.matmul(
                                                        den[:],
                                                        ones_s[:, 0:1],
                                                        pt[:],
                                                        start=(cc == 0),
                                                        stop=(cc == nchunks - 1),
                                                    )
                                                    nc.tensor.matmul(
                                                        po[:],
                                                        v_s[:, cc, g * 128:(g + 1) * 128],
                                                        pt[:],
                                                        start=(cc == 0),
                                                        stop=(cc == nchunks - 1),
                                                    )
                                                emit(denpv)

                                            def finalize(den=den, po=po, hh=hh, jj=jj):
                                                rec = smallp.tile([1, 512], F32R, tag="rec")
                                                with nc.allow_low_precision(
                                                    reason="f32r softmax recip"
                                                ):
                                                    nc.vector.reciprocal(rec[:], den[:])
                                                pb = dps.tile([128, 512], F32,
                                                              tag="pb", bufs=1)
                                                nc.tensor.matmul(
                                                    pb[:], ones_s[0:1, :], rec[:],
                                                    start=True, stop=True,
                                                )
                                                bs = smallp.tile([128, 512], F32, tag="bs")
                                                nc.vector.tensor_copy(bs[:], pb[:])
                                                ysl = y_s[:, hh:hh + 2,
                                                          jj * 256:(jj + 1) * 256]
                                                nc.vector.tensor_mul(
                                                    ysl,
                                                    po[:].rearrange(
                                                        "p (h q) -> p h q", h=2
                                                    ),
                                                    bs[:].rearrange(
                                                        "p (h q) -> p h q", h=2
                                                    ),
                                                )
                                            emit(finalize)

                                while deferred:
                                    deferred.popleft()()

                            # -------- Stage D: out^T = WoT.T @ y^T -----------
                            with tc.tile_pool(name="wo", bufs=3) as wop, \
                                 tc.tile_pool(name="oout", bufs=2) as ooutp, \
                                 tc.tile_pool(name="opsum2", bufs=3, space="PSUM") as ops2:
                                for oc in range(NCC):
                                    wo_strip = wop.tile([128, NCC, 128], F32R, tag="wo")
                                    nc.sync.dma_start(
                                        out=wo_strip[:],
                                        in_=woT.ap()[:, oc * 128:(oc + 1) * 128].rearrange(
                                            "(c p) m -> p c m", p=128
                                        ),
                                    )
                                    pso = ops2.tile([128, R], F32, tag="pso")
                                    for c in range(NCC):
                                        for rb in range(R // 512):
                                            nc.tensor.matmul(
                                                pso[:, rb * 512:(rb + 1) * 512],
                                                wo_strip[:, c, :],
                                                y_s[:, c, rb * 512:(rb + 1) * 512],
                                                start=(c == 0),
                                                stop=(c == NCC - 1),
                                            )
                                    ot = ooutp.tile([128, R], F32, tag="ot")
                                    nc.scalar.copy(ot[:], pso[:])
                                    nc.gpsimd.dma_start(
                                        out=outT.ap()[oc * 128:(oc + 1) * 128, :],
                                        in_=ot[:],
                                    )

                    cmaskp.release()
                    xt0p.release()

    nc.finalize()
    return nc


_NC_CACHE = None


def get_nc():
    global _NC_CACHE
    if _NC_CACHE is None:
        _NC_CACHE = _build_nc()
    return _NC_CACHE


def build_nrep(nrep):
    return _build_nc(nrep=nrep)


# Head-dim permutation: rope pair (i, i+64) -> partitions (32g+j, 32g+16+j)
# with g = i // 16, j = i % 16, so rotate-half is a quadrant-local swap.
_PERM = np.zeros(128, dtype=np.int64)   # new partition p holds old feature _PERM[p]
for _i in range(64):
    _g, _j = divmod(_i, 16)
    _PERM[32 * _g + _j] = _i
    _PERM[32 * _g + 16 + _j] = _i + 64


def _qpos(parity):
    """Global query row indices (length R) for a core with given parity."""
    blocks = np.arange(NQB) * 2 + parity          # global 128-blocks
    return (blocks[:, None] * 128 + np.arange(128)[None, :]).reshape(-1)


def _trig_tables(offset):
    inv_freq = 1.0 / (ROPE_BASE ** (np.arange(0, HD, 2, dtype=np.float64) / HD))
    pos = np.arange(offset, offset + T, dtype=np.float64)
    ang = pos[:, None] * inv_freq[None, :]        # [T, 64]
    cos = np.cos(ang).T                           # [64, T]
    sin = np.sin(ang).T
    cosT = np.empty((HD, T), dtype=np.float32)
    sinT = np.empty((HD, T), dtype=np.float32)
    for i in range(64):
        g, j = divmod(i, 16)
        cosT[32 * g + j] = cos[i]
        cosT[32 * g + 16 + j] = cos[i]
        sinT[32 * g + j] = -sin[i]                # sign-folded
        sinT[32 * g + 16 + j] = sin[i]
    return cosT, sinT


def _mask_table(parity):
    """Additive mask [128, 4*512]: shared across pair-blocks jj (the pattern
    only depends on relative position). Each 256-wide block is duplicated to
    512 so one DVE add covers the two-head-paired [128, 512] score tile."""
    m = np.zeros((128, 4 * 512), dtype=np.float32)
    ki = np.arange(128)
    q = np.arange(256)
    qrel = (parity + 2 * (q // 128)) * 128 + (q % 128)   # [256]
    for cb in range(4):
        kpos = cb * 128 + ki
        blk = np.where(kpos[:, None] <= qrel[None, :], 0.0, MASK_NEG)
        base = cb * 512
        m[:, base:base + 256] = blk
        m[:, base + 256:base + 512] = blk
    return m


def _permute_heads(w):
    """Permute the per-head 128 output-feature columns of a [C, n*128]
    transposed weight by _PERM."""
    nheads = w.shape[1] // HD
    w = w.reshape(w.shape[0], nheads, HD)
    return np.ascontiguousarray(w[:, :, _PERM].reshape(w.shape[0], nheads * HD))


def make_in_maps(x, Wq, Wk, Wv, Wo, offset):
    x = np.asarray(x, dtype=np.float32)
    Wq = np.asarray(Wq, dtype=np.float32)
    Wk = np.asarray(Wk, dtype=np.float32)
    Wv = np.asarray(Wv, dtype=np.float32)
    Wo = np.asarray(Wo, dtype=np.float32)
    offset = int(np.asarray(offset))

    scale = 1.0 / math.sqrt(HD)
    wqT = _permute_heads(np.ascontiguousarray((Wq * scale).T))  # [C, C]
    wkT = _permute_heads(np.ascontiguousarray(Wk.T))            # [C, 512]
    wvT = np.ascontiguousarray(Wv.T)
    woT = np.ascontiguousarray(Wo.T)
    cosT, sinT = _trig_tables(offset)
    ones = np.ones((128, 128), dtype=np.float32)

    in_maps = []
    for core in range(8):
        b, parity = core // 2, core % 2
        qpos = _qpos(parity)
        xb = x[b]                                   # [T, C]
        in_maps.append({
            "xT": np.ascontiguousarray(xb.T),
            "xqT": np.ascontiguousarray(xb[qpos].T),
            "wqT": wqT, "wkT": wkT, "wvT": wvT, "woT": woT,
            "cosq": np.ascontiguousarray(cosT[:, qpos]),
            "sinq": np.ascontiguousarray(sinT[:, qpos]),
            "cosk": cosT, "sink": sinT,
            "maskadd": _mask_table(parity),
            "ones_d": ones,
        })
    return in_maps


def assemble_output(results):
    out = np.empty((B, T, C), dtype=np.float32)
    for core in range(8):
        b, parity = core // 2, core % 2
        out[b, _qpos(parity), :] = results[core]["outT"].T
    return out


def kernel(x, Wq, Wk, Wv, Wo, offset):
    nc = get_nc()
    in_maps = make_in_maps(x, Wq, Wk, Wv, Wo, offset)
    res = run_bass_kernel_spmd(nc, in_maps, core_ids=list(range(8)))
    return assemble_output(res.results)


# revision 47
# speedup vs baseline: 3.4047x; 3.4047x over previous
"""Trainium2 Bass kernel for nn_MultiHeadAttention (GQA + RoPE + causal softmax).

Problem (hardcoded): B=4, T=2048, C=2048, n_head=16, n_kv_head=4, head_dim=128,
fp32 in/out, rope base 10000, torch-Linear style projections (x @ W.T).

Sharding: 8 cores = (4 batches) x (2 query shards). Each core handles one batch
and 1024 query rows picked as interleaved 128-row blocks (core parity 0 takes
even blocks, parity 1 odd blocks) so both cores of a batch run an identical
instruction stream (SPMD) with identical causal work. K/V are computed for the
full sequence on both cores of a pair. No collectives; host gathers outputs.

All matmuls run in float32r (TF32-like, ~1.5e-4 rel err). Device layout is
transposed ([feature, token]) so every matmul contraction is on partitions.

RoPE trick: the head_dim rows of Wq/Wk (and the trig tables) are permuted on
the host so each rotate-half pair (i, i+64) lands 16 partitions apart inside
one 32-partition quadrant. rotate_half then is a single DVE stream_shuffle
(quadrant-local 16<->16 swap) instead of cross-partition DMA copies. Dot
products q.k are invariant to the shared permutation.

Attention is flash-style per (kv-group, head-pair, 256-query block) with the
softmax-denominator and P.V matmuls deferred two chunks behind the score
matmuls so the PE never waits on the mask(DVE)+exp(ACT) latency. The
normalized output is written straight into an SBUF-resident y tile that the
output projection consumes directly (no DRAM roundtrip for y).
"""

import sys
import math
from collections import deque

sys.path.insert(0, "/opt/trn_rl_repo")

import numpy as np

import concourse.bacc as bacc
import concourse.mybir as mybir
import concourse.tile as tile
from concourse.bass_utils import run_bass_kernel_spmd

F32 = mybir.dt.float32
F32R = mybir.dt.float32r
BF16 = mybir.dt.bfloat16
AF = mybir.ActivationFunctionType

B, T, C = 4, 2048, 2048
NH, NKV, HD = 16, 4, 128
NREP = NH // NKV              # 4 q-heads per kv head
ROPE_BASE = 10000.0
R = T // 2                    # 1024 query rows per core
NCC = C // 128                # 16 contraction chunks
NQB = R // 128                # 8 local query blocks per core
NPAIR = NQB // 2              # 4 pair-blocks of 256 queries
MASK_NEG = -30000.0
SWAP_MASK = list(range(16, 32)) + list(range(16))
LOOKAHEAD = 4                 # slots of den/PV deferral behind S matmuls


def _build_nc(nrep=1):
    nc = bacc.Bacc(trn_type="TRN2", name="mha_gqa_rope")

    xT = nc.dram_tensor("xT", [C, T], BF16, kind="ExternalInput")
    xqT = nc.dram_tensor("xqT", [C, R], BF16, kind="ExternalInput")
    wqT = nc.dram_tensor("wqT", [NH // 2, 128, NCC, 256], BF16, kind="ExternalInput")
    wkT = nc.dram_tensor("wkT", [C, NKV * HD], BF16, kind="ExternalInput")
    wvT = nc.dram_tensor("wvT", [C, NKV * HD], BF16, kind="ExternalInput")
    woT = nc.dram_tensor("woT", [128, NCC, NCC * 128], BF16, kind="ExternalInput")
    cosq = nc.dram_tensor("cosq", [HD, R], F32, kind="ExternalInput")
    sinq = nc.dram_tensor("sinq", [HD, R], F32, kind="ExternalInput")
    cosk = nc.dram_tensor("cosk", [HD, T], F32, kind="ExternalInput")
    sink = nc.dram_tensor("sink", [HD, T], F32, kind="ExternalInput")
    maskadd = nc.dram_tensor("maskadd", [128, 4 * 512], F32, kind="ExternalInput")
    ones_d = nc.dram_tensor("ones_d", [128, 128], F32R, kind="ExternalInput")
    ones_bd = nc.dram_tensor("ones_bd", [128, 128], BF16, kind="ExternalInput")
    outT = nc.dram_tensor("outT", [C, R], F32, kind="ExternalOutput")

    with tile.TileContext(nc) as tc:
        with tc.tile_pool(name="dscr", bufs=1, space="DRAM") as dscr, \
             tc.tile_pool(name="const", bufs=1) as constp:
            qscr = dscr.tile([C, R], BF16)

            ones_s = constp.tile([128, 128], F32R)
            nc.sync.dma_start(out=ones_s[:], in_=ones_d.ap())
            ones_bs = constp.tile([128, 128], BF16)
            nc.sync.dma_start(out=ones_bs[:], in_=ones_bd.ap())

            for _rep in range(nrep):
                # K/V weights + key trig prefetched on the ACT (scalar) DMA
                # queue; transfers complete during stage Q. Explicit alloc /
                # release: these pools close after stage KV while kv_res
                # (opened later) persists into the attention stage.
                xt0p = tc.alloc_tile_pool(name="xt0", bufs=1, side="right")
                kvwp = tc.alloc_tile_pool(name="kvw", bufs=1, side="right")
                ktrigp = tc.alloc_tile_pool(name="ktrig", bufs=1, side="right")
                if True:
                    wk_s = kvwp.tile([128, NCC, NKV * HD], BF16, tag="wk")
                    wv_s = kvwp.tile([128, NCC, NKV * HD], BF16, tag="wv")
                    cosk_s = ktrigp.tile([HD, T], F32)
                    sink_s = ktrigp.tile([HD, T], F32)

                    # ------------- Stage Q: Q'^T = rope(WqT.T @ xqT) -> qscr --
                    with tc.tile_pool(name="xq", bufs=1) as xqp, \
                         tc.tile_pool(name="wq", bufs=2) as wqp, \
                         tc.tile_pool(name="qtrig", bufs=1) as qtrigp, \
                         tc.tile_pool(name="qrope", bufs=2) as qrp, \
                         tc.tile_pool(name="qpsum", bufs=3, space="PSUM") as qps:
        # Loads split across both HWDGE queues: sync carries wq
                        # strips + even xq chunks; ACT carries trig + odd xq
                        # chunks, then the stage-KV prefetches (wk/wv/cosk/sink).
                        xq_s = xqp.tile([128, NCC, R], BF16)
                        wq_strips = []

                        def load_wq(sp, split=1):
                            wq_strip = wqp.tile([128, NCC, 256], BF16, tag="wq",
                                                name=f"wq_strip{sp}")
                            step = NCC // split
                            for i in range(split):
                                csl = slice(i * step, (i + 1) * step)
                                nc.sync.dma_start(
                                    out=wq_strip[:, csl, :], in_=wqT.ap()[sp, :, csl]
                                )
                            wq_strips.append(wq_strip)

                        def load_xq(c, eng):
                            eng.dma_start(
                                out=xq_s[:, c, :],
                                in_=xqT.ap()[c * 128:(c + 1) * 128, :],
                            )

                        cosq_s = qtrigp.tile([HD, R], F32)
                        nc.scalar.dma_start(out=cosq_s[:], in_=cosq.ap())
                        sinq_s = qtrigp.tile([HD, R], F32)
                        nc.scalar.dma_start(out=sinq_s[:], in_=sinq.ap())
                        # first quarter of strip-pair 0, then x chunk 0, so the
                        # very first matmul's operands land first
                        wq_strip0 = wqp.tile([128, NCC, 256], BF16, tag="wq",
                                             name="wq_strip0")
                        nc.sync.dma_start(out=wq_strip0[:, 0:4, :],
                                          in_=wqT.ap()[0, :, 0:4])
                        load_xq(0, nc.sync)
                        for i in range(1, 4):
                            csl = slice(i * 4, (i + 1) * 4)
                            nc.sync.dma_start(out=wq_strip0[:, csl, :],
                                              in_=wqT.ap()[0, :, csl])
                        wq_strips.append(wq_strip0)
                        for c in range(NCC):
                            if c > 0:
                                load_xq(c, nc.sync if c % 2 == 0 else nc.scalar)
                            if c == 3:
                                load_wq(1)
                        # stage-KV prefetches ride the ACT queue from here
                        nc.scalar.dma_start(
                            out=wk_s[:], in_=wkT.ap().rearrange("(c p) k -> p c k", p=128)
                        )
                        nc.scalar.dma_start(
                            out=wv_s[:], in_=wvT.ap().rearrange("(c p) k -> p c k", p=128)
                        )
                        nc.scalar.dma_start(out=cosk_s[:], in_=cosk.ap())
                        nc.scalar.dma_start(out=sink_s[:], in_=sink.ap())
                        for qc in range(NH):  # 16 head-chunks of Q output dims
                            if qc % 2 == 0 and qc // 2 + 2 < NH // 2:
                                load_wq(qc // 2 + 2)
                            wq_strip = wq_strips[qc // 2][:, :, (qc % 2) * 128:
                                                          (qc % 2 + 1) * 128]
                            psqs = []
                            for rb in range(R // 512):
                                psq = qps.tile([128, 512], F32, tag="psq",
                                               name=f"psq{qc}_{rb}")
                                psqs.append(psq)
                            for c in range(NCC):
                                for rb in range(R // 512):
                                    nc.tensor.matmul(
                                        psqs[rb][:],
                                        wq_strip[:, c],
                                        xq_s[:, c, rb * 512:(rb + 1) * 512],
                                        start=(c == 0),
                                        stop=(c == NCC - 1),
                                    )
                            # rope via quadrant-local stream_shuffle (DVE only)
                            for rb in range(R // 512):
                                sl = slice(rb * 512, (rb + 1) * 512)
                                psq = psqs[rb]
                                rot = qrp.tile([128, 512], F32, tag="rot")
                                nc.vector.stream_shuffle(rot[:], psq[:], SWAP_MASK)
                                t1 = qrp.tile([128, 512], F32, tag="t1")
                                nc.vector.tensor_mul(t1[:], psq[:], cosq_s[:, sl])
                                nc.vector.tensor_mul(rot[:], rot[:], sinq_s[:, sl])
                                qf = qrp.tile([128, 512], BF16, tag="qf")
                                nc.vector.tensor_add(qf[:], t1[:], rot[:])
                                nc.gpsimd.dma_start(
                                    out=qscr[qc * 128:(qc + 1) * 128, sl], in_=qf[:]
                                )

                    # Causal-mask table: load early on the sync queue so it's
                    # resident well before the first attention mask-add.
                    cmaskp = tc.alloc_tile_pool(name="cmask", bufs=1)
                    mask_s = cmaskp.tile([128, 4 * 512], F32)
                    nc.scalar.dma_start(out=mask_s[:], in_=maskadd.ap())

                    # Full Wo stays resident (bf16): the output projection is
                    # interleaved into the attention stream per pair-block.
                    woallp = tc.alloc_tile_pool(name="woall", bufs=1)
                    wo_all = woallp.tile([128, NCC, NCC * 128], BF16)
                    nc.scalar.dma_start(out=wo_all[:], in_=woT.ap())

                    # First x block in a dedicated region (disjoint from the
                    # stage-Q pools) so its load isn't WAR-gated on stage Q.
                    xt0_tile = xt0p.tile([128, NCC, 256], BF16, tag="xt0", bufs=1)
                    nc.sync.dma_start(
                        out=xt0_tile[:],
                        in_=xT.ap()[:, 0:256].rearrange("(c p) t -> p c t", p=128),
                    )

                    # ------------- Stage KV ----------------------------------
                    with tc.tile_pool(name="kv_res", bufs=1) as kvres:
                        kT_s = kvres.tile([128, NKV, T], BF16)   # [d, g, t]
                        v_s = kvres.tile([128, T // 128, NKV * HD], BF16)

                        with tc.tile_pool(name="xt", bufs=2) as xtp, \
                             tc.tile_pool(name="krope", bufs=3) as krp, \
                             tc.tile_pool(name="kpsum", bufs=3, space="PSUM") as kps, \
                             tc.tile_pool(name="vpsum", bufs=2, space="PSUM") as vps:
                            for tb in range(T // 256):
                                if tb == 0:
                                    xt = xt0_tile
                                else:
                                    xt = xtp.tile([128, NCC, 256], BF16, tag="xt")
                                    nc.sync.dma_start(
                                        out=xt[:],
                                        in_=xT.ap()[:, tb * 256:(tb + 1) * 256].rearrange(
                                            "(c p) t -> p c t", p=128
                                        ),
                                    )
                                for g in range(NKV):
                                    psk = kps.tile([128, 256], F32, tag="psk")
                                    for c in range(NCC):
                                        nc.tensor.matmul(
                                            psk[:],
                                            wk_s[:, c, g * 128:(g + 1) * 128],
                                            xt[:, c, :],
                                            start=(c == 0),
                                            stop=(c == NCC - 1),
                                        )
                                    sl = slice(tb * 256, (tb + 1) * 256)
                                    rot = krp.tile([128, 256], F32, tag="krot")
                                    nc.vector.stream_shuffle(rot[:], psk[:], SWAP_MASK)
                                    t1 = krp.tile([128, 256], F32, tag="kt1")
                                    nc.vector.tensor_mul(t1[:], psk[:], cosk_s[:, sl])
                                    nc.vector.tensor_mul(rot[:], rot[:], sink_s[:, sl])
                                    nc.vector.tensor_add(kT_s[:, g, sl], t1[:], rot[:])
                                for ti in range(2):
                                    tchunk = tb * 2 + ti
                                    psv = vps.tile([128, NKV * HD], F32, tag="psv")
                                    for c in range(NCC):
                                        nc.tensor.matmul(
                                            psv[:],
                                            xt[:, c, ti * 128:(ti + 1) * 128],
                                            wv_s[:, c, :],
                                            start=(c == 0),
                                            stop=(c == NCC - 1),
                                        )
                                    nc.scalar.copy(v_s[:, tchunk, :], psv[:])

                        # qp for pair-block 0 reuses the xt0 region (free after
                        # t-block 0): its load runs during stage KV instead of
                        # being WAR-gated on the whole KV x ring.
                        qp0 = xt0p.tile([128, NH, 256], BF16, tag="xt0", bufs=1,
                                        name="qp0")
                        nc.sync.dma_start(
                            out=qp0[:],
                            in_=qscr[:, 0:256].rearrange("(h p) q -> p h q", p=128),
                        )

                        # K/V weights + key trig no longer needed
                        ktrigp.release()
                        kvwp.release()

                        # ---------- Stage C+D: attention + out proj ----------
                        with tc.tile_pool(name="ybuf", bufs=1) as ybufp:
                            y_s = ybufp.tile([128, NH, R], BF16)  # resident y^T

                            with tc.tile_pool(name="qp", bufs=2) as qpp, \
                                 tc.tile_pool(name="ptile", bufs=4) as ppp, \
                                 tc.tile_pool(name="small", bufs=2) as smallp, \
                                 tc.tile_pool(name="oout", bufs=3) as ooutp, \
                                 tc.tile_pool(name="spsum", bufs=3, space="PSUM") as sps, \
                                 tc.tile_pool(name="opsum", bufs=2, space="PSUM") as ops, \
                                 tc.tile_pool(name="opsum2", bufs=1, space="PSUM") as ops2, \
                                 tc.tile_pool(name="dpsum", bufs=2, space="PSUM") as dps:
                                deferred = deque()

                                def emit(fn):
                                    deferred.append(fn)
                                    while len(deferred) > LOOKAHEAD:
                                        deferred.popleft()()

                                for jj in range(NPAIR):
                                    if jj == 0:
                                        qp = qp0
                                    else:
                                        qp = qpp.tile([128, NH, 256], BF16, tag="qp")
                                        nc.sync.dma_start(
                                            out=qp[:],
                                            in_=qscr[:, jj * 256:(jj + 1) * 256].rearrange(
                                                "(h p) q -> p h q", p=128
                                            ),
                                        )
                                    qp_flat = qp[:].rearrange("p h q -> p (h q)")
                                    nchunks = 4 * jj + 4
                                    for g in range(NKV):
                                        # the two head-pairs of a kv group are
                                        # interleaved chunk-by-chunk so den/PV
                                        # trail the S matmuls by a full chunk
                                        dens = [dps.tile([128, 512], F32, tag="den",
                                                         name=f"den{jj}_{g}_{h}")
                                                for h in range(2)]
                                        pos = [ops.tile([128, 512], F32, tag="po",
                                                        name=f"po{jj}_{g}_{h}")
                                               for h in range(2)]
                                        for cc in range(nchunks):
                                            for hp in range(NREP // 2):
                                                hh = g * NREP + hp * 2
                                                den, po = dens[hp], pos[hp]
                                                pss = sps.tile([128, 512], F32, tag="pss",
                                                               bufs=3)
                                                nc.tensor.matmul(
                                                    pss[:],
                                                    kT_s[:, g, cc * 128:(cc + 1) * 128],
                                                    qp_flat[:, hh * 256:(hh + 2) * 256],
                                                    start=True,
                                                    stop=True,
                                                )
                                                if cc >= 4 * jj:
                                                    moff = (cc - 4 * jj) * 512
                                                    nc.vector.tensor_add(
                                                        pss[:], pss[:],
                                                        mask_s[:, moff:moff + 512],
                                                    )
                                                pt = ppp.tile([128, 512], BF16, tag="pt")
                                                nc.scalar.activation(pt[:], pss[:], AF.Exp)

                                                def denpv(den=den, po=po, pt=pt,
                                                          cc=cc, nchunks=nchunks, g=g):
                                                    nc.tensor.matmul(
                                                        den[:],
                                                        ones_bs[:],
                                                        pt[:],
                                                        start=(cc == 0),
                                                        stop=(cc == nchunks - 1),
                                                    )
                                                    nc.tensor.matmul(
                                                        po[:],
                                                        v_s[:, cc, g * 128:(g + 1) * 128],
                                                        pt[:],
                                                        start=(cc == 0),
                                                        stop=(cc == nchunks - 1),
                                                    )
                                                emit(denpv)

                                        for hp in range(NREP // 2):
                                            hh = g * NREP + hp * 2
                                            den, po = dens[hp], pos[hp]

                                            def finalize(den=den, po=po, hh=hh, jj=jj):
                                                bs = smallp.tile([128, 512], F32, tag="bs")
                                                nc.vector.reciprocal_approx_fast(
                                                    out=bs[:], in_=den[:]
                                                )
                                                ysl = y_s[:, hh:hh + 2,
                                                          jj * 256:(jj + 1) * 256]
                                                nc.vector.tensor_mul(
                                                    ysl,
                                                    po[:].rearrange(
                                                        "p (h q) -> p h q", h=2
                                                    ),
                                                    bs[:].rearrange(
                                                        "p (h q) -> p h q", h=2
                                                    ),
                                                )
                                            emit(finalize)

                                    # output projection for this pair-block,
                                    # interleaved into the deferred stream so
                                    # its matmuls fill exp-latency stalls of
                                    # the next pair-block
                                    for oc in range(NCC):
                                        def opiece(oc=oc, jj=jj):
                                            if jj == NPAIR - 1:
                                                # tail drains with no attention
                                                # work left; alternate two banks
                                                # from the freed po ring
                                                pso = ops.tile([128, 256], F32,
                                                               tag="po")
                                            else:
                                                pso = ops2.tile([128, 256], F32,
                                                                tag="pso")
                                            qsl = slice(jj * 256, (jj + 1) * 256)
                                            for c in range(NCC):
                                                nc.tensor.matmul(
                                                    pso[:],
                                                    wo_all[:, c,
                                                           oc * 128:(oc + 1) * 128],
                                                    y_s[:, c, qsl],
                                                    start=(c == 0),
                                                    stop=(c == NCC - 1),
                                                )
                                            ot = ooutp.tile([128, 256], F32,
                                                            tag="ot")
                                            nc.scalar.copy(ot[:], pso[:])
                                            nc.gpsimd.dma_start(
                                                out=outT.ap()[
                                                    oc * 128:(oc + 1) * 128, qsl
                                                ],
                                                in_=ot[:],
                                            )
                                        emit(opiece)

                                while deferred:
                                    deferred.popleft()()

                    woallp.release()
                    cmaskp.release()
                    xt0p.release()

    nc.finalize()
    return nc


_NC_CACHE = None


def get_nc():
    global _NC_CACHE
    if _NC_CACHE is None:
        _NC_CACHE = _build_nc()
    return _NC_CACHE


def build_nrep(nrep):
    return _build_nc(nrep=nrep)


# Head-dim permutation: rope pair (i, i+64) -> partitions (32g+j, 32g+16+j)
# with g = i // 16, j = i % 16, so rotate-half is a quadrant-local swap.
_PERM = np.zeros(128, dtype=np.int64)   # new partition p holds old feature _PERM[p]
for _i in range(64):
    _g, _j = divmod(_i, 16)
    _PERM[32 * _g + _j] = _i
    _PERM[32 * _g + 16 + _j] = _i + 64


def _qpos(parity):
    """Global query row indices (length R) for a core with given parity."""
    blocks = np.arange(NQB) * 2 + parity          # global 128-blocks
    return (blocks[:, None] * 128 + np.arange(128)[None, :]).reshape(-1)


def _trig_tables(offset):
    inv_freq = 1.0 / (ROPE_BASE ** (np.arange(0, HD, 2, dtype=np.float64) / HD))
    pos = np.arange(offset, offset + T, dtype=np.float64)
    ang = pos[:, None] * inv_freq[None, :]        # [T, 64]
    cos = np.cos(ang).T                           # [64, T]
    sin = np.sin(ang).T
    cosT = np.empty((HD, T), dtype=np.float32)
    sinT = np.empty((HD, T), dtype=np.float32)
    for i in range(64):
        g, j = divmod(i, 16)
        cosT[32 * g + j] = cos[i]
        cosT[32 * g + 16 + j] = cos[i]
        sinT[32 * g + j] = -sin[i]                # sign-folded
        sinT[32 * g + 16 + j] = sin[i]
    return cosT, sinT


def _mask_table(parity):
    """Additive mask [128, 4*512]: shared across pair-blocks jj (the pattern
    only depends on relative position). Each 256-wide block is duplicated to
    512 so one DVE add covers the two-head-paired [128, 512] score tile."""
    m = np.zeros((128, 4 * 512), dtype=np.float32)
    ki = np.arange(128)
    q = np.arange(256)
    qrel = (parity + 2 * (q // 128)) * 128 + (q % 128)   # [256]
    for cb in range(4):
        kpos = cb * 128 + ki
        blk = np.where(kpos[:, None] <= qrel[None, :], 0.0, MASK_NEG)
        base = cb * 512
        m[:, base:base + 256] = blk
        m[:, base + 256:base + 512] = blk
    return m


def _permute_heads(w):
    """Permute the per-head 128 output-feature columns of a [C, n*128]
    transposed weight by _PERM."""
    nheads = w.shape[1] // HD
    w = w.reshape(w.shape[0], nheads, HD)
    return np.ascontiguousarray(w[:, :, _PERM].reshape(w.shape[0], nheads * HD))


def _strip_major(wT):
    """[C, n*128] transposed weight -> [n, 128, NCC, 128] matching the SBUF
    strip layout [partition, c-chunk, m], so strip loads are contiguous."""
    n = wT.shape[1] // 128
    return np.ascontiguousarray(
        wT.reshape(NCC, 128, n, 128).transpose(2, 1, 0, 3)
    )


def make_in_maps(x, Wq, Wk, Wv, Wo, offset):
    bf16 = mybir.dt.np(BF16)
    x = np.asarray(x, dtype=np.float32)
    Wq = np.asarray(Wq, dtype=np.float32)
    Wk = np.asarray(Wk, dtype=np.float32)
    Wv = np.asarray(Wv, dtype=np.float32)
    Wo = np.asarray(Wo, dtype=np.float32)
    offset = int(np.asarray(offset))

    scale = 1.0 / math.sqrt(HD)
    wq_strips = _strip_major(
        _permute_heads(np.ascontiguousarray((Wq * scale).T))
    )                                                           # [NH,128,NCC,128]
    wqT = np.ascontiguousarray(
        wq_strips.reshape(NH // 2, 2, 128, NCC, 128)
        .transpose(0, 2, 3, 1, 4).reshape(NH // 2, 128, NCC, 256)
    ).astype(bf16)                                              # paired strips
    wkT = _permute_heads(np.ascontiguousarray(Wk.T)).astype(bf16)  # [C, 512]
    wvT = np.ascontiguousarray(Wv.T).astype(bf16)
    woT = np.ascontiguousarray(
        Wo.T.reshape(NCC, 128, NCC, 128).transpose(1, 0, 2, 3)
        .reshape(128, NCC, NCC * 128)
    ).astype(bf16)                                  # SBUF layout [p, c, (o m)]
    cosT, sinT = _trig_tables(offset)
    ones = np.ones((128, 128), dtype=np.float32)

    in_maps = []
    for core in range(8):
        b, parity = core // 2, core % 2
        qpos = _qpos(parity)
        xb = x[b]                                   # [T, C]
        in_maps.append({
            "xT": np.ascontiguousarray(xb.T.astype(bf16)),
            "xqT": np.ascontiguousarray(xb[qpos].T.astype(bf16)),
            "wqT": wqT, "wkT": wkT, "wvT": wvT, "woT": woT,
            "cosq": np.ascontiguousarray(cosT[:, qpos]),
            "sinq": np.ascontiguousarray(sinT[:, qpos]),
            "cosk": cosT, "sink": sinT,
            "maskadd": _mask_table(parity),
            "ones_d": ones,
            "ones_bd": ones.astype(bf16),
        })
    return in_maps


def assemble_output(results):
    out = np.empty((B, T, C), dtype=np.float32)
    for core in range(8):
        b, parity = core // 2, core % 2
        out[b, _qpos(parity), :] = results[core]["outT"].T
    return out


def kernel(x, Wq, Wk, Wv, Wo, offset):
    nc = get_nc()
    in_maps = make_in_maps(x, Wq, Wk, Wv, Wo, offset)
    res = run_bass_kernel_spmd(nc, in_maps, core_ids=list(range(8)))
    return assemble_output(res.results)
